# revision 2
# baseline (speedup 1.0000x reference)
"""Trainium2 Bass kernel for DirectHorizontalLineFilter.

Reference computation (per [H, W] image, B*C images):
  vs   = 5-tap vertical box filter of x (replicate pad)      [H, W]
  std  = per-row std over W (ddof=1)                         [H, 1]
  m    = sigmoid((0.05 - std) * 10)                          [H, 1]
  mf   = 5-tap vertical box filter of m (replicate pad)      [H, 1]
  w    = 0.8 * mf
  out  = x * (1 - w) + vs * w  ==  x + w * (vs - x)

Sharding: batch dim (8) across 8 cores, data parallel, no collectives.

Per-core plan (64 channels of [512, 512] fp32, natural row-major layout,
H rows on SBUF partitions):
  - 5 halo-overlapped row tiles per channel (rows r0..r0+K), so every tile
    is self-contained for the +-2 vertical stencil.
  - bn_stats/bn_aggr (DVE) -> per-row variance; ACT chain (batched over the
    5 tiles of a channel): std = sqrt(var*512/511), m = sigmoid(0.5-10*std).
  - The vertical 5-tap filter is a banded-matrix fp32 matmul on PE:
    lhsT = 0.8*B for the mask column (STRENGTH folded into the weights) and
    B - I for the image, with replicate-pad edge weights baked into the
    first/last tile's matrix.  vs' = (B-I)^T x, w = (0.8 B)^T m, so the
    blend is out = x + w*vs'.
  - ACT does the PSUM->SBUF copy fused with the per-row w scale
    (Copy activation with a per-partition scale AP); the final
    out = (w*vs') + x add runs on GPSIMD (tiles 0-3) and DVE (tile 4),
    keeping DVE/ACT/Pool/PE all below the HBM roofline.
  - Channels are processed in groups of 8 with phase-ordered emission so
    ACT table switches (Sqrt set <-> Exp set) happen 2x per group
    instead of 2x per channel; sigmoid's tail is computed exactly as
    1/(1+exp(.)) with a DVE reciprocal.
  - DMA: one 1 MiB halo-overlapped load covers tiles 0-3 of a channel
    (raw strided AP), one merged store covers rows 126..497; loads
    alternate between the SP and ACT HWDGE queues by channel parity.
"""

import numpy as np
from contextlib import ExitStack

import concourse.bacc as bacc
import concourse.bass as bass
import concourse.tile as tile
import concourse.mybir as mybir
from concourse.bass_utils import run_bass_kernel_spmd

B, C, H, W = 8, 64, 512, 512
N_CORES = 8
F32 = mybir.dt.float32
F32R = mybir.dt.float32r
# float32r runs the image-filter matmul at 4x PE throughput; the DMA-produced
# f32r x is rounded to ~tf32, giving ~2.4e-4 worst-case relative error on the
# x passthrough (vs ~5e-8 at fp32).  Flip to False for exact fp32 matmuls.
USE_F32R = True
XDT = F32R if USE_F32R else F32
AF = mybir.ActivationFunctionType
OP = mybir.AluOpType

STRENGTH = 0.8
THRESHOLD = 0.05
GROUP = 8  # channels per phase-group

# (r0, K, p0, p1): load rows r0..r0+K, output rows r0+p0..r0+p1-1 on
# partitions p0..p1-1 (partition index == row - r0 everywhere).
GRID = [
    (0, 128, 0, 126),
    (124, 128, 2, 126),
    (248, 128, 2, 126),
    (372, 128, 2, 126),
    (496, 16, 2, 16),
]


def _filter_matrices():
    """Banded 5-tap matrices per tile: Bm (mask filter, scaled by STRENGTH)
    and Bp = B - I (image filter).

    All K output columns are populated (matmul writes partitions [0, K) so
    its PSUM base partition is 0); columns outside [p0, p1) produce values
    that are never read — the band is simply clipped at the tile edge.
    """
    bms, bps = [], []
    for r0, K, p0, p1 in GRID:
        b = np.zeros((K, K), np.float32)
        for m in range(K):
            h = r0 + m
            for d in (-2, -1, 0, 1, 2):
                k = min(max(h + d, 0), H - 1) - r0
                if 0 <= k < K:
                    b[k, m] += np.float32(0.2)
        bp = b.copy()
        for m in range(K):
            bp[m, m] -= np.float32(1.0)
        bms.append(b * np.float32(STRENGTH))
        bps.append(bp)
    return bms, bps


_CACHE = {}


def _build(use_f32r=None, split_loads=True, loop_iters=None):
    # loop_iters: benchmarking aid — wraps the whole body in a tc.For_i so
    # one NEFF execution runs the kernel loop_iters times (idempotent).
    if use_f32r is None:
        use_f32r = USE_F32R
    key = ("nc", use_f32r, split_loads, loop_iters)
    if key in _CACHE:
        return _CACHE[key]
    xdt = F32R if use_f32r else F32

    nc = bacc.Bacc(
        "TRN2", target_bir_lowering=False, debug=False, num_devices=N_CORES
    )
    x_ap = nc.dram_tensor("x", [C, H, W], xdt, kind="ExternalInput").ap()
    y_ap = nc.dram_tensor("y", [C, H, W], F32, kind="ExternalOutput").ap()
    wm_aps, wp_aps = [], []
    for t, (r0, K, p0, p1) in enumerate(GRID):
        wm_aps.append(
            nc.dram_tensor(f"wm{t}", [K, K], F32, kind="ExternalInput").ap()
        )
        wp_aps.append(
            nc.dram_tensor(f"wp{t}", [K, K], xdt, kind="ExternalInput").ap()
        )

    NT = len(GRID)

    with tile.TileContext(nc) as tc, ExitStack() as ctx:
        wpool = ctx.enter_context(tc.tile_pool(name="weights", bufs=1))
        xpool = ctx.enter_context(tc.tile_pool(name="x", bufs=GROUP + 2))
        vpool = ctx.enter_context(tc.tile_pool(name="vs_sb", bufs=6))
        opool = ctx.enter_context(tc.tile_pool(name="out", bufs=3))
        spool = ctx.enter_context(tc.tile_pool(name="stats", bufs=GROUP + 4))
        psum_vs = ctx.enter_context(
            tc.tile_pool(name="psum_vs", bufs=6, space="PSUM")
        )
        psum_w = ctx.enter_context(
            tc.tile_pool(name="psum_w", bufs=2, space="PSUM")
        )

        wm_tiles, wp_tiles = [], []
        for t, (r0, K, p0, p1) in enumerate(GRID):
            wm = wpool.tile([K, K], F32, tag=f"wm{t}")
            nc.sync.dma_start(out=wm[:], in_=wm_aps[t])
            wm_tiles.append(wm)
            wp = wpool.tile([K, K], xdt, tag=f"wp{t}")
            nc.sync.dma_start(out=wp[:], in_=wp_aps[t])
            wp_tiles.append(wp)

        exp_bias = wpool.tile([128, 1], F32, tag="exp_bias")
        nc.vector.memset(exp_bias[:], -10.0 * THRESHOLD)

        loop_cm = (
            tc.For_i(0, loop_iters, 1) if loop_iters is not None else None
        )
        if loop_cm is not None:
            loop_cm.__enter__()

        for g0 in range(0, C, GROUP):
            chans = list(range(g0, min(g0 + GROUP, C)))
            G = len(chans)
            xts, wsbs = {}, {}

            # phase 1: loads
            for c in chans:
                xts[c] = []
                ld_eng = nc.sync if (c % 2 == 0 or not split_loads) else nc.scalar
                # one 1 MiB halo-overlapped DMA covers tiles 0-3
                xbig = xpool.tile([128, 4 * W], xdt, tag="xbig")
                src_ap = bass.AP(
                    x_ap.tensor, c * H * W,
                    [[W, 128], [124 * W, 4], [1, W]],
                )
                ld_eng.dma_start(
                    out=xbig[:].rearrange("p (t w) -> p t w", t=4), in_=src_ap
                )
                for t in range(4):
                    xts[c].append(xbig[:, t * W : (t + 1) * W])
                r0, K, p0, p1 = GRID[4]
                x4 = xpool.tile([128, W], xdt, tag="x4")
                ld_eng.dma_start(out=x4[0:K, :], in_=x_ap[c, r0 : r0 + K, :])
                xts[c].append(x4[:, :])

            # phase 2: row stats (DVE) -> one shared per-group aggr tile so
            # the sqrt/exp chain below runs once per group (2 ACT table
            # loads per group instead of 2 per channel)
            gaggr = spool.tile([128, 2 * NT * GROUP], F32, tag="gaggr")
            nc.gpsimd.memset(gaggr[:], 0.0)
            for cl, c in enumerate(chans):
                stats = spool.tile([128, 6 * NT], F32, tag="stats")
                for t, (r0, K, p0, p1) in enumerate(GRID):
                    nc.vector.bn_stats(
                        out=stats[0:K, 6 * t : 6 * t + 6],
                        in_=xts[c][t][0:K].bitcast(F32),
                    )
                    j = 2 * (cl * NT + t)
                    nc.vector.bn_aggr(
                        out=gaggr[0:K, j : j + 2],
                        in_=stats[0:K, 6 * t : 6 * t + 6],
                    )

            # phase 3 (batched over the whole group):
            #   std = sqrt(var_pop * N/(N-1))          (ACT Sqrt table)
            #   e   = exp(10*std - 0.5)                (ACT Exp table)
            #   m   = 1/(1+e) = sigmoid(0.5 - 10*std)  (DVE, exact tail)
            stdb = spool.tile([128, NT * GROUP], F32, tag="stdb")
            var_view = gaggr[:].rearrange("p (g two) -> p g two", two=2)[:, :, 1]
            nc.scalar.activation(
                out=stdb[:, 0 : NT * G], in_=var_view[:, 0 : NT * G],
                func=AF.Sqrt, scale=float(W) / (W - 1),
            )
            expb = spool.tile([128, NT * GROUP], F32, tag="expb")
            nc.scalar.activation(
                out=expb[:, 0 : NT * G], in_=stdb[:, 0 : NT * G],
                func=AF.Exp, bias=exp_bias[:], scale=10.0,
            )
            mpre = spool.tile([128, NT * GROUP], F32, tag="mpre")
            nc.vector.tensor_scalar_add(
                mpre[:, 0 : NT * G], expb[:, 0 : NT * G], 1.0
            )
            nc.vector.reciprocal(mpre[:, 0 : NT * G], mpre[:, 0 : NT * G])

            # phase 4: mask filter matmuls (PE) + w PSUM->SBUF copy (ACT)
            for cl, c in enumerate(chans):
                wfp = psum_w.tile([128, 8], F32, tag="wfp")
                # the 16-row tile's matmul only writes partitions [0,16) of
                # its column; define the rest for the full-tile copy below
                nc.vector.memset(wfp[:, NT - 1 : NT], 0.0)
                for t, (r0, K, p0, p1) in enumerate(GRID):
                    j = cl * NT + t
                    nc.tensor.matmul(
                        out=wfp[0:K, t : t + 1],
                        lhsT=wm_tiles[t][0:K, 0:K],
                        rhs=mpre[0:K, j : j + 1],
                        start=True, stop=True,
                    )
                w_sb = spool.tile([128, NT], F32, tag="w_sb")
                nc.scalar.copy(out=w_sb[:], in_=wfp[:, 0:NT])
                wsbs[c] = w_sb

            # phase 5: image filter matmul (PE), vs' PSUM->SBUF (ACT),
            # blend on GPSIMD (all-SBUF STT), store
            for c in chans:
                obig = opool.tile([128, 3 * W], F32, tag="obig")
                for t, (r0, K, p0, p1) in enumerate(GRID):
                    vsp = psum_vs.tile([128, W], F32, tag="vs")
                    nc.tensor.matmul(
                        out=vsp[0:K, :],
                        lhsT=wp_tiles[t][0:K, 0:K],
                        rhs=xts[c][t][0:K],
                        start=True, stop=True,
                    )
                    # fused PSUM->SBUF copy with the per-row w scale (ACT),
                    # then out = w*vs' + x as a plain add on GPSIMD
                    vs_sb = vpool.tile([128, W], F32, tag="vs_sb")
                    nc.scalar.activation(
                        out=vs_sb[0:K, :], in_=vsp[0:K, :], func=AF.Copy,
                        scale=wsbs[c][0:K, t : t + 1],
                    )
                    if t in (1, 2, 3):
                        ot = obig[:, (t - 1) * W : t * W]
                    else:
                        otile = opool.tile(
                            [128, W], F32, tag=f"ot{t}", name=f"ot{t}"
                        )
                        ot = otile[:, :]
                    # balance the final adds: the 16-row tile's add on DVE,
                    # the full tiles on GPSIMD
                    add_eng = nc.vector if t == NT - 1 else nc.gpsimd
                    add_eng.tensor_tensor(
                        out=ot[0:K],
                        in0=vs_sb[0:K, :],
                        in1=xts[c][t][0:K].bitcast(F32),
                        op=OP.add,
                    )
                    if t == 0:
                        nc.sync.dma_start(
                            out=y_ap[c, 0:126, :], in_=ot[0:126]
                        )
                    elif t == NT - 1:
                        nc.sync.dma_start(
                            out=y_ap[c, r0 + p0 : r0 + p1, :], in_=ot[p0:p1]
                        )
                # tiles 1-3 (rows 126..497) as three 2D stores: a single
                # merged 3D store with a partition-offset source lands all
                # its packets on SDMA engines 64-67 only (HW descriptor
                # fan-out pathology, confirmed by trace), capping store
                # bandwidth at ~84 GB/s; 2D stores spread over all 16.
                for bblk in range(3):
                    nc.sync.dma_start(
                        out=y_ap[c, 126 + 124 * bblk : 250 + 124 * bblk, :],
                        in_=obig[2:126, bblk * W : (bblk + 1) * W],
                    )

        if loop_cm is not None:
            loop_cm.__exit__(None, None, None)

    nc.compile()
    _CACHE[key] = nc
    return nc


def kernel(x: np.ndarray) -> np.ndarray:
    assert x.shape == (B, C, H, W), x.shape
    nc = _build()
    bms, bps = _filter_matrices()
    in_maps = []
    for i in range(N_CORES):
        m = {"x": np.ascontiguousarray(x[i], dtype=np.float32)}
        for t in range(len(GRID)):
            m[f"wm{t}"] = bms[t]
            m[f"wp{t}"] = bps[t]
        in_maps.append(m)
    res = run_bass_kernel_spmd(nc, in_maps, list(range(N_CORES)))
    out = np.stack([res.results[i]["y"] for i in range(N_CORES)], axis=0)
    return out.astype(np.float32)



# revision 6
# speedup vs baseline: 3.6792x; 3.6792x over previous
"""Trainium2 Bass kernel for DirectHorizontalLineFilter.

Reference computation (per [H, W] image, B*C images):
  vs   = 5-tap vertical box filter of x (replicate pad)      [H, W]
  std  = per-row std over W (ddof=1)                         [H, 1]
  m    = sigmoid((0.05 - std) * 10)                          [H, 1]
  mf   = 5-tap vertical box filter of m (replicate pad)      [H, 1]
  w    = 0.8 * mf
  out  = x * (1 - w) + vs * w  ==  x + w * (vs - x)

Sharding: batch dim (8) across 8 cores, data parallel, no collectives.

Per-core plan (64 channels of [512, 512] fp32, natural row-major layout,
H rows on SBUF partitions):
  - 5 halo-overlapped row tiles per channel (rows r0..r0+K), so every tile
    is self-contained for the +-2 vertical stencil.
  - bn_stats/bn_aggr (DVE) -> per-row variance; ACT chain (batched over the
    5 tiles of a channel): std = sqrt(var*512/511), m = sigmoid(0.5-10*std).
  - The vertical 5-tap filter is a banded-matrix fp32 matmul on PE:
    lhsT = 0.8*B for the mask column (STRENGTH folded into the weights) and
    B - I for the image, with replicate-pad edge weights baked into the
    first/last tile's matrix.  vs' = (B-I)^T x, w = (0.8 B)^T m, so the
    blend is out = x + w*vs'.
  - ACT does the PSUM->SBUF copy fused with the per-row w scale
    (Copy activation with a per-partition scale AP); the final
    out = (w*vs') + x add runs on GPSIMD (tiles 0-3) and DVE (tile 4),
    keeping DVE/ACT/Pool/PE all below the HBM roofline.
  - Channels are processed in groups of 8 with phase-ordered emission so
    ACT table switches (Sqrt set <-> Exp set) happen 2x per group
    instead of 2x per channel; sigmoid's tail is computed exactly as
    1/(1+exp(.)) with a DVE reciprocal.
  - DMA: one 1 MiB halo-overlapped load covers tiles 0-3 of a channel
    (raw strided AP), one merged store covers rows 126..497; loads
    alternate between the SP and ACT HWDGE queues by channel parity.
"""

import numpy as np
from contextlib import ExitStack

import concourse.bacc as bacc
import concourse.bass as bass
import concourse.tile as tile
import concourse.mybir as mybir
from concourse.bass_utils import run_bass_kernel_spmd

B, C, H, W = 8, 64, 512, 512
N_CORES = 8
F32 = mybir.dt.float32
F32R = mybir.dt.float32r
# float32r runs the image-filter matmul at 4x PE throughput; the DMA-produced
# f32r x is rounded to ~tf32, giving ~2.4e-4 worst-case relative error on the
# x passthrough (vs ~5e-8 at fp32).  Flip to False for exact fp32 matmuls.
USE_F32R = True
XDT = F32R if USE_F32R else F32
AF = mybir.ActivationFunctionType
OP = mybir.AluOpType

STRENGTH = 0.8
THRESHOLD = 0.05
GROUP = 8  # channels per phase-group

# (r0, K, p0, p1): load rows r0..r0+K, output rows r0+p0..r0+p1-1 on
# partitions p0..p1-1 (partition index == row - r0 everywhere).
GRID = [
    (0, 128, 0, 126),
    (124, 128, 2, 126),
    (248, 128, 2, 126),
    (372, 128, 2, 126),
    (496, 16, 2, 16),
]


def _filter_matrices():
    """Banded 5-tap matrices per tile: Bm (mask filter, scaled by STRENGTH)
    and Bp = B - I (image filter).

    All K output columns are populated (matmul writes partitions [0, K) so
    its PSUM base partition is 0); columns outside [p0, p1) produce values
    that are never read — the band is simply clipped at the tile edge.
    """
    bms, bps = [], []
    for r0, K, p0, p1 in GRID:
        b = np.zeros((K, K), np.float32)
        for m in range(K):
            h = r0 + m
            for d in (-2, -1, 0, 1, 2):
                k = min(max(h + d, 0), H - 1) - r0
                if 0 <= k < K:
                    b[k, m] += np.float32(0.2)
        bp = b.copy()
        for m in range(K):
            bp[m, m] -= np.float32(1.0)
        bms.append(b * np.float32(STRENGTH))
        bps.append(bp)
    return bms, bps


_CACHE = {}


def _build(use_f32r=None, split_loads=True, loop_iters=None):
    # loop_iters: benchmarking aid — wraps the whole body in a tc.For_i so
    # one NEFF execution runs the kernel loop_iters times (idempotent).
    if use_f32r is None:
        use_f32r = USE_F32R
    key = ("nc", use_f32r, split_loads, loop_iters)
    if key in _CACHE:
        return _CACHE[key]
    xdt = F32R if use_f32r else F32

    nc = bacc.Bacc(
        "TRN2", target_bir_lowering=False, debug=False, num_devices=N_CORES
    )
    x_ap = nc.dram_tensor("x", [C, H, W], xdt, kind="ExternalInput").ap()
    y_ap = nc.dram_tensor("y", [C, H, W], F32, kind="ExternalOutput").ap()
    wm_aps, wp_aps = [], []
    for t, (r0, K, p0, p1) in enumerate(GRID):
        wm_aps.append(
            nc.dram_tensor(f"wm{t}", [K, K], F32, kind="ExternalInput").ap()
        )
        wp_aps.append(
            nc.dram_tensor(f"wp{t}", [K, K], xdt, kind="ExternalInput").ap()
        )

    NT = len(GRID)

    with tile.TileContext(nc) as tc, ExitStack() as ctx:
        wpool = ctx.enter_context(tc.tile_pool(name="weights", bufs=1))
        xpool = ctx.enter_context(tc.tile_pool(name="x", bufs=GROUP + 2))
        vpool = ctx.enter_context(tc.tile_pool(name="vs_sb", bufs=3))
        opool = ctx.enter_context(tc.tile_pool(name="out", bufs=3))
        spool = ctx.enter_context(tc.tile_pool(name="stats", bufs=GROUP + 4))
        psum_vs = ctx.enter_context(
            tc.tile_pool(name="psum_vs", bufs=6, space="PSUM")
        )
        psum_w = ctx.enter_context(
            tc.tile_pool(name="psum_w", bufs=2, space="PSUM")
        )

        wm_tiles, wp_tiles = [], []
        for t, (r0, K, p0, p1) in enumerate(GRID):
            wm = wpool.tile([K, K], F32, tag=f"wm{t}")
            nc.sync.dma_start(out=wm[:], in_=wm_aps[t])
            wm_tiles.append(wm)
            wp = wpool.tile([K, K], xdt, tag=f"wp{t}")
            nc.sync.dma_start(out=wp[:], in_=wp_aps[t])
            wp_tiles.append(wp)

        exp_bias = wpool.tile([128, 1], F32, tag="exp_bias")
        nc.vector.memset(exp_bias[:], -10.0 * THRESHOLD)

        loop_cm = (
            tc.For_i(0, loop_iters, 1) if loop_iters is not None else None
        )
        if loop_cm is not None:
            loop_cm.__enter__()

        for g0 in range(0, C, GROUP):
            chans = list(range(g0, min(g0 + GROUP, C)))
            G = len(chans)
            xts, wsbs, xbigs = {}, {}, {}

            # phase 1: loads
            for c in chans:
                xts[c] = []
                ld_eng = nc.sync if (c % 2 == 0 or not split_loads) else nc.scalar
                # one 1 MiB halo-overlapped DMA covers tiles 0-3
                xbig = xpool.tile([128, 4 * W], xdt, tag="xbig")
                src_ap = bass.AP(
                    x_ap.tensor, c * H * W,
                    [[W, 128], [124 * W, 4], [1, W]],
                )
                ld_eng.dma_start(
                    out=xbig[:].rearrange("p (t w) -> p t w", t=4), in_=src_ap
                )
                xbigs[c] = xbig
                for t in range(4):
                    xts[c].append(xbig[:, t * W : (t + 1) * W])
                r0, K, p0, p1 = GRID[4]
                x4 = xpool.tile([128, W], xdt, tag="x4")
                ld_eng.dma_start(out=x4[0:K, :], in_=x_ap[c, r0 : r0 + K, :])
                xts[c].append(x4[:, :])

            # phase 2: row stats (DVE) -> one shared per-group aggr tile so
            # the sqrt/exp chain below runs once per group (2 ACT table
            # loads per group instead of 2 per channel)
            gaggr = spool.tile([128, 2 * NT * GROUP], F32, tag="gaggr")
            nc.gpsimd.memset(gaggr[:], 0.0)
            for cl, c in enumerate(chans):
                stats = spool.tile([128, 6 * NT], F32, tag="stats")
                for t, (r0, K, p0, p1) in enumerate(GRID):
                    nc.vector.bn_stats(
                        out=stats[0:K, 6 * t : 6 * t + 6],
                        in_=xts[c][t][0:K].bitcast(F32),
                    )
                    j = 2 * (cl * NT + t)
                    nc.vector.bn_aggr(
                        out=gaggr[0:K, j : j + 2],
                        in_=stats[0:K, 6 * t : 6 * t + 6],
                    )

            # phase 3 (batched over the whole group):
            #   std = sqrt(var_pop * N/(N-1))          (ACT Sqrt table)
            #   e   = exp(10*std - 0.5)                (ACT Exp table)
            #   m   = 1/(1+e) = sigmoid(0.5 - 10*std)  (DVE, exact tail)
            stdb = spool.tile([128, NT * GROUP], F32, tag="stdb")
            var_view = gaggr[:].rearrange("p (g two) -> p g two", two=2)[:, :, 1]
            nc.scalar.activation(
                out=stdb[:, 0 : NT * G], in_=var_view[:, 0 : NT * G],
                func=AF.Sqrt, scale=float(W) / (W - 1),
            )
            expb = spool.tile([128, NT * GROUP], F32, tag="expb")
            nc.scalar.activation(
                out=expb[:, 0 : NT * G], in_=stdb[:, 0 : NT * G],
                func=AF.Exp, bias=exp_bias[:], scale=10.0,
            )
            mpre = spool.tile([128, NT * GROUP], F32, tag="mpre")
            nc.vector.tensor_scalar_add(
                mpre[:, 0 : NT * G], expb[:, 0 : NT * G], 1.0
            )
            nc.vector.reciprocal(mpre[:, 0 : NT * G], mpre[:, 0 : NT * G])

            # phase 4: mask filter matmuls (PE) + w PSUM->SBUF copy (ACT)
            for cl, c in enumerate(chans):
                wfp = psum_w.tile([128, 8], F32, tag="wfp")
                # the 16-row tile's matmul only writes partitions [0,16) of
                # its column; define the rest for the full-tile copy below
                nc.vector.memset(wfp[:, NT - 1 : NT], 0.0)
                for t, (r0, K, p0, p1) in enumerate(GRID):
                    j = cl * NT + t
                    nc.tensor.matmul(
                        out=wfp[0:K, t : t + 1],
                        lhsT=wm_tiles[t][0:K, 0:K],
                        rhs=mpre[0:K, j : j + 1],
                        start=True, stop=True,
                    )
                w_sb = spool.tile([128, NT], F32, tag="w_sb")
                nc.scalar.copy(out=w_sb[:], in_=wfp[:, 0:NT])
                wsbs[c] = w_sb

            # phase 5: image filter matmul (PE), vs' PSUM->SBUF (ACT),
            # blend on GPSIMD (all-SBUF STT), store
            for c in chans:
                obig = opool.tile([128, 3 * W], F32, tag="obig")
                vsbig = vpool.tile([128, 3 * W], F32, tag="vsbig")
                for t, (r0, K, p0, p1) in enumerate(GRID):
                    vsp = psum_vs.tile([128, W], F32, tag="vs")
                    nc.tensor.matmul(
                        out=vsp[0:K, :],
                        lhsT=wp_tiles[t][0:K, 0:K],
                        rhs=xts[c][t][0:K],
                        start=True, stop=True,
                    )
                    # fused PSUM->SBUF copy with the per-row w scale (ACT),
                    # then out = w*vs' + x as a plain add on GPSIMD
                    if t in (1, 2, 3):
                        vs_sb = vsbig[:, (t - 1) * W : t * W]
                    else:
                        vtile = vpool.tile([128, W], F32, tag=f"vs{t}")
                        vs_sb = vtile[:, :]
                    nc.scalar.activation(
                        out=vs_sb[0:K, :], in_=vsp[0:K, :], func=AF.Copy,
                        scale=wsbs[c][0:K, t : t + 1],
                    )
                    if t == 0:
                        otile = opool.tile([128, W], F32, tag="ot0")
                        nc.gpsimd.tensor_tensor(
                            out=otile[0:K],
                            in0=vs_sb[0:K, :],
                            in1=xts[c][t][0:K].bitcast(F32),
                            op=OP.add,
                        )
                        nc.sync.dma_start(
                            out=y_ap[c, 0:126, :], in_=otile[0:126]
                        )
                    elif t == NT - 1:
                        otile = opool.tile([128, W], F32, tag="ot4")
                        nc.vector.tensor_tensor(
                            out=otile[0:K],
                            in0=vs_sb[0:K, :],
                            in1=xts[c][t][0:K].bitcast(F32),
                            op=OP.add,
                        )
                        nc.sync.dma_start(
                            out=y_ap[c, r0 + p0 : r0 + p1, :],
                            in_=otile[p0:p1],
                        )
                # one wide blend add for tiles 1-3 (amortizes the ~0.9us
                # per-instruction GPSIMD overhead over 3W columns)
                nc.gpsimd.tensor_tensor(
                    out=obig[:, :],
                    in0=vsbig[:, :],
                    in1=xbigs[c][:, W : 4 * W].bitcast(F32),
                    op=OP.add,
                )
                # tiles 1-3 (rows 126..497): HWDGE fans a store across
                # SDMA engines = largest divisor <= 16 of the partition
                # count. 124 rows -> only 4 engines (124 = 4*31), which
                # caps store bandwidth at ~84 GB/s and was the kernel's
                # bottleneck. Split at partition 66: 64 rows -> 16 engines,
                # 60 rows -> 15 engines.
                for pa, pb in ((2, 66), (66, 126)):
                    dst_ap = bass.AP(
                        y_ap.tensor, c * H * W + (124 + pa) * W,
                        [[W, pb - pa], [124 * W, 3], [1, W]],
                    )
                    nc.sync.dma_start(
                        out=dst_ap,
                        in_=obig[pa:pb, :].rearrange(
                            "p (t w) -> p t w", t=3
                        ),
                    )

        if loop_cm is not None:
            loop_cm.__exit__(None, None, None)

    nc.compile()
    _CACHE[key] = nc
    return nc


def kernel(x: np.ndarray) -> np.ndarray:
    assert x.shape == (B, C, H, W), x.shape
    nc = _build()
    bms, bps = _filter_matrices()
    in_maps = []
    for i in range(N_CORES):
        m = {"x": np.ascontiguousarray(x[i], dtype=np.float32)}
        for t in range(len(GRID)):
            m[f"wm{t}"] = bms[t]
            m[f"wp{t}"] = bps[t]
        in_maps.append(m)
    res = run_bass_kernel_spmd(nc, in_maps, list(range(N_CORES)))
    out = np.stack([res.results[i]["y"] for i in range(N_CORES)], axis=0)
    return out.astype(np.float32)



# revision 7
# speedup vs baseline: 3.7904x; 1.0302x over previous
"""Trainium2 Bass kernel for DirectHorizontalLineFilter.

Reference computation (per [H, W] image, B*C images):
  vs   = 5-tap vertical box filter of x (replicate pad)      [H, W]
  std  = per-row std over W (ddof=1)                         [H, 1]
  m    = sigmoid((0.05 - std) * 10)                          [H, 1]
  mf   = 5-tap vertical box filter of m (replicate pad)      [H, 1]
  w    = 0.8 * mf
  out  = x * (1 - w) + vs * w  ==  x + w * (vs - x)

Sharding: batch dim (8) across 8 cores, data parallel, no collectives.

Per-core plan (64 channels of [512, 512] fp32, natural row-major layout,
H rows on SBUF partitions):
  - 5 halo-overlapped row tiles per channel (rows r0..r0+K), so every tile
    is self-contained for the +-2 vertical stencil.
  - bn_stats/bn_aggr (DVE) -> per-row variance; ACT chain (batched over the
    5 tiles of a channel): std = sqrt(var*512/511), m = sigmoid(0.5-10*std).
  - The vertical 5-tap filter is a banded-matrix fp32 matmul on PE:
    lhsT = 0.8*B for the mask column (STRENGTH folded into the weights) and
    B - I for the image, with replicate-pad edge weights baked into the
    first/last tile's matrix.  vs' = (B-I)^T x, w = (0.8 B)^T m, so the
    blend is out = x + w*vs'.
  - ACT does the PSUM->SBUF copy fused with the per-row w scale
    (Copy activation with a per-partition scale AP); the final
    out = (w*vs') + x add runs on GPSIMD (tiles 0-3) and DVE (tile 4),
    keeping DVE/ACT/Pool/PE all below the HBM roofline.
  - Channels are processed in groups of 8 with phase-ordered emission so
    ACT table switches (Sqrt set <-> Exp set) happen 2x per group
    instead of 2x per channel; sigmoid's tail is computed exactly as
    1/(1+exp(.)) with a DVE reciprocal.
  - DMA: one 1 MiB halo-overlapped load covers tiles 0-3 of a channel
    (raw strided AP), one merged store covers rows 126..497; loads
    alternate between the SP and ACT HWDGE queues by channel parity.
"""

import numpy as np
from contextlib import ExitStack

import concourse.bacc as bacc
import concourse.bass as bass
import concourse.tile as tile
import concourse.mybir as mybir
from concourse.bass_utils import run_bass_kernel_spmd

B, C, H, W = 8, 64, 512, 512
N_CORES = 8
F32 = mybir.dt.float32
F32R = mybir.dt.float32r
# float32r runs the image-filter matmul at 4x PE throughput; the DMA-produced
# f32r x is rounded to ~tf32, giving ~2.4e-4 worst-case relative error on the
# x passthrough (vs ~5e-8 at fp32).  Flip to False for exact fp32 matmuls.
USE_F32R = True
XDT = F32R if USE_F32R else F32
AF = mybir.ActivationFunctionType
OP = mybir.AluOpType

STRENGTH = 0.8
THRESHOLD = 0.05
GROUP = 4  # channels per phase-group

# (r0, K, p0, p1): load rows r0..r0+K, output rows r0+p0..r0+p1-1 on
# partitions p0..p1-1 (partition index == row - r0 everywhere).
GRID = [
    (0, 128, 0, 126),
    (124, 128, 2, 126),
    (248, 128, 2, 126),
    (372, 128, 2, 126),
    (496, 16, 2, 16),
]


def _filter_matrices():
    """Banded 5-tap matrices per tile: Bm (mask filter, scaled by STRENGTH)
    and Bp = B - I (image filter).

    All K output columns are populated (matmul writes partitions [0, K) so
    its PSUM base partition is 0); columns outside [p0, p1) produce values
    that are never read — the band is simply clipped at the tile edge.
    """
    bms, bps = [], []
    for r0, K, p0, p1 in GRID:
        b = np.zeros((K, K), np.float32)
        for m in range(K):
            h = r0 + m
            for d in (-2, -1, 0, 1, 2):
                k = min(max(h + d, 0), H - 1) - r0
                if 0 <= k < K:
                    b[k, m] += np.float32(0.2)
        bp = b.copy()
        for m in range(K):
            bp[m, m] -= np.float32(1.0)
        bms.append(b * np.float32(STRENGTH))
        bps.append(bp)
    return bms, bps


_CACHE = {}


def _build(use_f32r=None, split_loads=True, loop_iters=None):
    # loop_iters: benchmarking aid — wraps the whole body in a tc.For_i so
    # one NEFF execution runs the kernel loop_iters times (idempotent).
    if use_f32r is None:
        use_f32r = USE_F32R
    key = ("nc", use_f32r, split_loads, loop_iters)
    if key in _CACHE:
        return _CACHE[key]
    xdt = F32R if use_f32r else F32

    nc = bacc.Bacc(
        "TRN2", target_bir_lowering=False, debug=False, num_devices=N_CORES
    )
    x_ap = nc.dram_tensor("x", [C, H, W], xdt, kind="ExternalInput").ap()
    y_ap = nc.dram_tensor("y", [C, H, W], F32, kind="ExternalOutput").ap()
    wm_aps, wp_aps = [], []
    for t, (r0, K, p0, p1) in enumerate(GRID):
        wm_aps.append(
            nc.dram_tensor(f"wm{t}", [K, K], F32, kind="ExternalInput").ap()
        )
        wp_aps.append(
            nc.dram_tensor(f"wp{t}", [K, K], xdt, kind="ExternalInput").ap()
        )

    NT = len(GRID)

    with tile.TileContext(nc) as tc, ExitStack() as ctx:
        wpool = ctx.enter_context(tc.tile_pool(name="weights", bufs=1))
        xpool = ctx.enter_context(tc.tile_pool(name="x", bufs=GROUP + 4))
        vpool = ctx.enter_context(tc.tile_pool(name="vs_sb", bufs=3))
        opool = ctx.enter_context(tc.tile_pool(name="out", bufs=3))
        spool = ctx.enter_context(tc.tile_pool(name="stats", bufs=GROUP + 4))
        psum_vs = ctx.enter_context(
            tc.tile_pool(name="psum_vs", bufs=6, space="PSUM")
        )
        psum_w = ctx.enter_context(
            tc.tile_pool(name="psum_w", bufs=2, space="PSUM")
        )

        wm_tiles, wp_tiles = [], []
        for t, (r0, K, p0, p1) in enumerate(GRID):
            wm = wpool.tile([K, K], F32, tag=f"wm{t}")
            nc.sync.dma_start(out=wm[:], in_=wm_aps[t])
            wm_tiles.append(wm)
            wp = wpool.tile([K, K], xdt, tag=f"wp{t}")
            nc.sync.dma_start(out=wp[:], in_=wp_aps[t])
            wp_tiles.append(wp)

        exp_bias = wpool.tile([128, 1], F32, tag="exp_bias")
        nc.vector.memset(exp_bias[:], -10.0 * THRESHOLD)

        loop_cm = (
            tc.For_i(0, loop_iters, 1) if loop_iters is not None else None
        )
        if loop_cm is not None:
            loop_cm.__enter__()

        for g0 in range(0, C, GROUP):
            chans = list(range(g0, min(g0 + GROUP, C)))
            G = len(chans)
            xts, wsbs, xbigs = {}, {}, {}

            # phase 1: loads
            for c in chans:
                xts[c] = []
                ld_eng = nc.sync if (c % 2 == 0 or not split_loads) else nc.scalar
                # one 1 MiB halo-overlapped DMA covers tiles 0-3
                xbig = xpool.tile([128, 4 * W], xdt, tag="xbig")
                src_ap = bass.AP(
                    x_ap.tensor, c * H * W,
                    [[W, 128], [124 * W, 4], [1, W]],
                )
                ld_eng.dma_start(
                    out=xbig[:].rearrange("p (t w) -> p t w", t=4), in_=src_ap
                )
                xbigs[c] = xbig
                for t in range(4):
                    xts[c].append(xbig[:, t * W : (t + 1) * W])
                r0, K, p0, p1 = GRID[4]
                x4 = xpool.tile([128, W], xdt, tag="x4")
                ld_eng.dma_start(out=x4[0:K, :], in_=x_ap[c, r0 : r0 + K, :])
                xts[c].append(x4[:, :])

            # phase 2: row stats (DVE) -> one shared per-group aggr tile so
            # the sqrt/exp chain below runs once per group (2 ACT table
            # loads per group instead of 2 per channel)
            gaggr = spool.tile([128, 2 * NT * GROUP], F32, tag="gaggr")
            nc.gpsimd.memset(gaggr[:], 0.0)
            for cl, c in enumerate(chans):
                stats = spool.tile([128, 6 * NT], F32, tag="stats")
                for t, (r0, K, p0, p1) in enumerate(GRID):
                    nc.vector.bn_stats(
                        out=stats[0:K, 6 * t : 6 * t + 6],
                        in_=xts[c][t][0:K].bitcast(F32),
                    )
                    j = 2 * (cl * NT + t)
                    nc.vector.bn_aggr(
                        out=gaggr[0:K, j : j + 2],
                        in_=stats[0:K, 6 * t : 6 * t + 6],
                    )

            # phase 3 (batched over the whole group):
            #   std = sqrt(var_pop * N/(N-1))          (ACT Sqrt table)
            #   e   = exp(10*std - 0.5)                (ACT Exp table)
            #   m   = 1/(1+e) = sigmoid(0.5 - 10*std)  (DVE, exact tail)
            stdb = spool.tile([128, NT * GROUP], F32, tag="stdb")
            var_view = gaggr[:].rearrange("p (g two) -> p g two", two=2)[:, :, 1]
            nc.scalar.activation(
                out=stdb[:, 0 : NT * G], in_=var_view[:, 0 : NT * G],
                func=AF.Sqrt, scale=float(W) / (W - 1),
            )
            expb = spool.tile([128, NT * GROUP], F32, tag="expb")
            nc.scalar.activation(
                out=expb[:, 0 : NT * G], in_=stdb[:, 0 : NT * G],
                func=AF.Exp, bias=exp_bias[:], scale=10.0,
            )
            mpre = spool.tile([128, NT * GROUP], F32, tag="mpre")
            nc.vector.tensor_scalar_add(
                mpre[:, 0 : NT * G], expb[:, 0 : NT * G], 1.0
            )
            nc.vector.reciprocal(mpre[:, 0 : NT * G], mpre[:, 0 : NT * G])

            # phase 4: mask filter matmuls (PE) + w PSUM->SBUF copy (ACT)
            for cl, c in enumerate(chans):
                wfp = psum_w.tile([128, 8], F32, tag="wfp")
                # the 16-row tile's matmul only writes partitions [0,16) of
                # its column; define the rest for the full-tile copy below
                nc.vector.memset(wfp[:, NT - 1 : NT], 0.0)
                for t, (r0, K, p0, p1) in enumerate(GRID):
                    j = cl * NT + t
                    nc.tensor.matmul(
                        out=wfp[0:K, t : t + 1],
                        lhsT=wm_tiles[t][0:K, 0:K],
                        rhs=mpre[0:K, j : j + 1],
                        start=True, stop=True,
                    )
                w_sb = spool.tile([128, NT], F32, tag="w_sb")
                nc.scalar.copy(out=w_sb[:], in_=wfp[:, 0:NT])
                wsbs[c] = w_sb

            # phase 5: image filter matmul (PE), vs' PSUM->SBUF (ACT),
            # blend on GPSIMD (all-SBUF STT), store
            for c in chans:
                obig = opool.tile([128, 3 * W], F32, tag="obig")
                vsbig = vpool.tile([128, 3 * W], F32, tag="vsbig")
                for t, (r0, K, p0, p1) in enumerate(GRID):
                    vsp = psum_vs.tile([128, W], F32, tag="vs")
                    nc.tensor.matmul(
                        out=vsp[0:K, :],
                        lhsT=wp_tiles[t][0:K, 0:K],
                        rhs=xts[c][t][0:K],
                        start=True, stop=True,
                    )
                    # fused PSUM->SBUF copy with the per-row w scale (ACT),
                    # then out = w*vs' + x as a plain add on GPSIMD
                    if t in (1, 2, 3):
                        vs_sb = vsbig[:, (t - 1) * W : t * W]
                    else:
                        vtile = vpool.tile([128, W], F32, tag=f"vs{t}")
                        vs_sb = vtile[:, :]
                    nc.scalar.activation(
                        out=vs_sb[0:K, :], in_=vsp[0:K, :], func=AF.Copy,
                        scale=wsbs[c][0:K, t : t + 1],
                    )
                    if t == 0:
                        otile = opool.tile([128, W], F32, tag="ot0")
                        nc.gpsimd.tensor_tensor(
                            out=otile[0:K],
                            in0=vs_sb[0:K, :],
                            in1=xts[c][t][0:K].bitcast(F32),
                            op=OP.add,
                        )
                        nc.sync.dma_start(
                            out=y_ap[c, 0:126, :], in_=otile[0:126]
                        )
                    elif t == NT - 1:
                        otile = opool.tile([128, W], F32, tag="ot4")
                        nc.vector.tensor_tensor(
                            out=otile[0:K],
                            in0=vs_sb[0:K, :],
                            in1=xts[c][t][0:K].bitcast(F32),
                            op=OP.add,
                        )
                        nc.sync.dma_start(
                            out=y_ap[c, r0 + p0 : r0 + p1, :],
                            in_=otile[p0:p1],
                        )
                # one wide blend add for tiles 1-3 (amortizes the ~0.9us
                # per-instruction GPSIMD overhead over 3W columns)
                nc.gpsimd.tensor_tensor(
                    out=obig[:, :],
                    in0=vsbig[:, :],
                    in1=xbigs[c][:, W : 4 * W].bitcast(F32),
                    op=OP.add,
                )
                # tiles 1-3 (rows 126..497): HWDGE fans a store across
                # SDMA engines = largest divisor <= 16 of the partition
                # count. 124 rows -> only 4 engines (124 = 4*31), which
                # caps store bandwidth at ~84 GB/s and was the kernel's
                # bottleneck. Split at partition 66: 64 rows -> 16 engines,
                # 60 rows -> 15 engines.
                for pa, pb in ((2, 66), (66, 126)):
                    dst_ap = bass.AP(
                        y_ap.tensor, c * H * W + (124 + pa) * W,
                        [[W, pb - pa], [124 * W, 3], [1, W]],
                    )
                    nc.sync.dma_start(
                        out=dst_ap,
                        in_=obig[pa:pb, :].rearrange(
                            "p (t w) -> p t w", t=3
                        ),
                    )

        if loop_cm is not None:
            loop_cm.__exit__(None, None, None)

    nc.compile()
    _CACHE[key] = nc
    return nc


def kernel(x: np.ndarray) -> np.ndarray:
    assert x.shape == (B, C, H, W), x.shape
    nc = _build()
    bms, bps = _filter_matrices()
    in_maps = []
    for i in range(N_CORES):
        m = {"x": np.ascontiguousarray(x[i], dtype=np.float32)}
        for t in range(len(GRID)):
            m[f"wm{t}"] = bms[t]
            m[f"wp{t}"] = bps[t]
        in_maps.append(m)
    res = run_bass_kernel_spmd(nc, in_maps, list(range(N_CORES)))
    out = np.stack([res.results[i]["y"] for i in range(N_CORES)], axis=0)
    return out.astype(np.float32)

